# revision 6
# baseline (speedup 1.0000x reference)
"""Single-head attention (b=4, s=4096, d=1024, h=128) on 8 Trainium2 NeuronCores.

Sharding: data-parallel over batch x query-halves -> 8 independent cores
(core c handles batch c//2, query rows [hq*2048, (hq+1)*2048) with hq = c%2).
K/V work is replicated per batch pair; no collectives.

Host prep per core: x[b].T (d-major) in bf16, sequence columns rotated so
the core's 2048 query rows come first (softmax over keys is permutation-
invariant); weights in bf16, pre-arranged to [128p, 8c*128h] rows so their
DMA runs 2KB-contiguous descriptors; 1/sqrt(h) folded into Wq. The kernel
returns outT [h, 2048] f32 per core; the host transposes back.

Device kernel v5 (bf16 matmul operands, fp32 PSUM accumulation),
software-pipelined so the PE never idles:

  unit (kb, h): sc = kT[kb].T @ qT[half] [128k, 1024q] PSUM f32 (2 MMs);
    ex = exp(sc) (one 1024-wide ACT instr); oT += v[kb].T' @ ex (PSUM
    accumulate); den chain acc[h] += ex on DVE (bf16).
  Units run h0 kb0..31 then h1 kb0..31 (sequential q-halves), so the oT
  accumulator needs only one 2-bank PSUM tile, reused across halves.
  The sc pool (2 bufs x 2 banks) serves ONLY score fronts -> front(k)
  waits exactly on exp(k-2); exp paces any pure-unit stretch at ~1.1us
  vs 0.86us of PE work per unit.
  That deficit is paid by the PIPELINE: the Q/K/V projections for the
  NEXT iteration accumulate in a dedicated 1-bank PSUM pool ("pp") and
  are emitted as fine matmul slices (~2.75 per unit) with the V chunks
  woven between the wider q/k matmuls (hides V's per-MM ldweights), so
  the PE always has filler and the iteration boundary has no bubble.
  Finales (den = ones.T @ acc via PE, DVE reciprocal+multiply, output
  DMA) are only emitted at pp-chain boundaries: allocating the den
  chunks while a sliced projection chain holds a pp slot would deadlock
  the in-order PE against the pool rotation.
  PSUM budget: oT 2 + sc 4 + pp 2 = 8 banks.

TimelineSim steady state: ~90.6us/body with PE 99.8% busy (engine-busy
91us, ACT 67us, DVE 55us), vs ~115us for the v2 plan this replaces.
The single-shot path (used by kernel()) runs prologue projections, then
one unit pass -- same instruction mix without the next-iteration filler.
"""

import sys

sys.path.insert(0, "/opt/trn_rl_repo")

import numpy as np
import ml_dtypes

import concourse.mybir as mybir
from concourse import bacc
from concourse.bass_utils import run_bass_kernel_spmd

F32 = mybir.dt.float32
BF16 = mybir.dt.bfloat16

B = 4  # batch
D = 1024  # d_model
H = 128  # head size
S = 4096  # full sequence (keys)
SQ = 2048  # queries per core
DC = D // 128  # 8 d-chunks
NB = S // 512  # 8 column blocks for projections
KB = S // 128  # 32 key blocks
SC_BUFS = 2
EXP_BUFS = 8
XT_BUFS = 16  # 2 nb-groups in flight; 64 allocs/body stays phase-aligned


def build_attention_v5(loop_n=None, timing=False, unroll=None):
    """Build the v5 attention kernel; see module docstring.

    loop_n (even) wraps a 2-body software-pipelined pair in an on-device
    For_i loop with Internal zero-filled inputs for steady-state timing.
    unroll=N emits prologue + N pipelined bodies without For_i (sim use).
    """
    from concourse.tile import TileContext

    nc = bacc.Bacc("TRN2", target_bir_lowering=False, debug=False)

    kind_in = "Internal" if timing else "ExternalInput"
    xT = nc.dram_tensor("xT", (D, S), BF16, kind=kind_in)
    # weights pre-arranged on host to [128p, DC*H] so the DMA runs 2KB rows
    wq = nc.dram_tensor("wq", (128, DC * H), BF16, kind=kind_in)
    wk = nc.dram_tensor("wk", (128, DC * H), BF16, kind=kind_in)
    wv = nc.dram_tensor("wv", (128, DC * H), BF16, kind=kind_in)
    outT = nc.dram_tensor("outT", (H, SQ), F32, kind="ExternalOutput")
    tick = (
        nc.dram_tensor("tick", (1, 16), F32, kind="ExternalInput") if timing else None
    )
    warm_d = nc.dram_tensor("warm_d", (1, 16), F32, kind="Internal")

    with TileContext(nc) as tc:
        with (
            tc.tile_pool(name="consts", bufs=1) as cpool,
            tc.tile_pool(name="big", bufs=2) as big,
            tc.tile_pool(name="xtp", bufs=XT_BUFS) as xtp,
            tc.tile_pool(name="expp", bufs=EXP_BUFS) as expp,
            tc.tile_pool(name="work", bufs=2) as work,
            tc.tile_pool(name="ps", bufs=1, space="PSUM") as ps,
        ):
            # ---- one-time prologue ----
            if timing:
                tick_sb = cpool.tile([1, 16], F32)
                nc.sync.dma_start(out=tick_sb, in_=tick[0:1, :])
                zs = cpool.tile([128, 2048], F32)
                nc.vector.memset(zs, 0.0)
                zs_bf = zs.bitcast(BF16)  # [128, 4096] bf16 zeros
                for pb in range(DC):
                    nc.sync.dma_start(out=xT[pb * 128 : (pb + 1) * 128, :], in_=zs_bf)
                for w in (wq, wk, wv):
                    nc.sync.dma_start(out=w[:, :], in_=zs_bf[:, : DC * H])

            wq_sb = cpool.tile([128, DC, H], BF16)
            nc.sync.dma_start(out=wq_sb, in_=wq.rearrange("p (c h) -> p c h", c=DC))
            wk_sb = cpool.tile([128, DC, H], BF16)
            nc.sync.dma_start(out=wk_sb, in_=wk.rearrange("p (c h) -> p c h", c=DC))
            wv_sb = cpool.tile([128, DC, H], BF16)
            nc.sync.dma_start(out=wv_sb, in_=wv.rearrange("p (c h) -> p c h", c=DC))
            ones_f32 = cpool.tile([128, 128], F32)
            nc.vector.memset(ones_f32, 1.0)
            ones_sb = cpool.tile([128, 128], BF16)
            nc.vector.tensor_copy(out=ones_sb, in_=ones_f32)
            # tiny dummy exp so the ACT table set loads before the first unit
            warm = cpool.tile([1, 16], F32)
            nc.scalar.activation(
                warm, ones_f32[0:1, 0:16], mybir.ActivationFunctionType.Exp
            )
            # warm must have a reader for the BIR verifier; park it in a
            # DRAM scratch nothing else touches
            nc.sync.dma_start(out=warm_d[0:1, :], in_=warm)

            # single oT accumulator, reused h0 -> h1 -> next body
            oT_ps = ps.tile([128, 1024], F32, tag="oT", bufs=1, name="oT")

            xt_tiles = {}

            def emit_xt(nb):
                xts = []
                for dc in range(DC):
                    xt_t = xtp.tile([128, 512], BF16, tag="xt", name=f"xt{nb}_{dc}")
                    nc.sync.dma_start(
                        out=xt_t,
                        in_=xT[dc * 128 : (dc + 1) * 128, nb * 512 : (nb + 1) * 512],
                    )
                    xts.append(xt_t)
                xt_tiles[nb] = xts

            def alloc_act(sfx):
                # static tiles (unique names, no pool rotation): the loop pair
                # alternates A/B explicitly, so slot machinery never wraps
                return {
                    "qT": big.tile([128, SQ], BF16, name=f"qT_{sfx}"),
                    "kT": big.tile([128, S], BF16, name=f"kT_{sfx}"),
                    "v": big.tile([128, KB, 128], BF16, name=f"v_{sfx}"),
                    "accs": [
                        big.tile([128, 1024], BF16, name=f"acc{h}_{sfx}")
                        for h in range(2)
                    ],
                }

            # ---- projection slices ----------------------------------------
            # One proj chain = one pp-pool tile ([128,512] f32, 1 bank) that
            # several MM slices accumulate into, then one DVE copy out.
            # Slices are closures so emit_body can interleave them freely.

            def make_qk_chain(act, kind, nb):
                st = {}

                def start():
                    if nb not in xt_tiles:
                        emit_xt(nb)
                    if nb + 1 < NB and nb + 1 not in xt_tiles:
                        emit_xt(nb + 1)
                    st["t"] = ps.tile(
                        [128, 512], F32, tag="pp", bufs=2, name=f"{kind}ps{nb}"
                    )

                w_sb = wq_sb if kind == "q" else wk_sb
                slices = []
                for s0 in range(0, DC, 2):

                    def sl(s0=s0):
                        if s0 == 0:
                            start()
                        for dc in (s0, s0 + 1):
                            nc.tensor.matmul(
                                st["t"],
                                w_sb[:, dc],
                                xt_tiles[nb][dc],
                                start=dc == 0,
                                stop=dc == DC - 1,
                            )
                        if s0 + 2 == DC:
                            dst = act["qT"] if kind == "q" else act["kT"]
                            nc.vector.tensor_copy(
                                out=dst[:, nb * 512 : (nb + 1) * 512], in_=st["t"]
                            )

                    slices.append(sl)
                return slices

            def make_v_chain(act, nb, chunk=DC):
                # v natural [k, h]: xt slices stationary, wv moving -- no
                # transpose pass needed for the PV stationary operand.
                # chunk = matmuls per slice (2 -> fine weave so the per-MM
                # ldweights loads hide under neighboring wider qk matmuls)
                st = {}
                slices = []
                for t in range(4):
                    for d0 in range(0, DC, chunk):

                        def sl(t=t, d0=d0):
                            if t == 0 and d0 == 0:
                                st["t"] = ps.tile(
                                    [128, 512], F32, tag="pp", bufs=2, name=f"vps{nb}"
                                )
                            vt = st["t"].rearrange("p (t h) -> p t h", t=4)
                            for dc in range(d0, min(d0 + chunk, DC)):
                                nc.tensor.matmul(
                                    vt[:, t],
                                    xt_tiles[nb][dc][:, t * 128 : (t + 1) * 128],
                                    wv_sb[:, dc],
                                    start=dc == 0,
                                    stop=dc == DC - 1,
                                )
                            if t == 3 and d0 + chunk >= DC:
                                nc.vector.tensor_copy(
                                    out=act["v"][:, nb * 4 : (nb + 1) * 4], in_=vt
                                )
                                xt_tiles.pop(nb)

                        slices.append(sl)
                return slices

            def proj_slices(act):
                """Returns (slices, opens): opens[i] = number of pp-pool
                chains still open AFTER slice i. Finale den chunks may only
                be emitted when the count is zero (all pp slots released ->
                no in-order PE deadlock against the pool rotation)."""
                slices = []
                opens = []
                for nb in range(NB):
                    chains = []
                    if nb < SQ // 512:
                        chains.append(make_qk_chain(act, "q", nb))
                    chains.append(make_qk_chain(act, "k", nb))
                    chains.append(make_v_chain(act, nb, chunk=2))
                    # weave v chunks evenly between q/k chunks; at most two
                    # chains (one qk + the v) hold pp slots at a time
                    qk = [c for ch in chains[:-1] for c in ch]
                    vch = chains[-1]
                    per = len(vch) // len(qk)
                    order = []
                    vi = 0
                    for i, c in enumerate(qk):
                        order.append(c)
                        take = per if i < len(qk) - 1 else len(vch) - vi
                        for _ in range(take):
                            order.append(vch[vi])
                            vi += 1
                    first = {id(ch[0]): ci for ci, ch in enumerate(chains)}
                    last = {id(ch[-1]): ci for ci, ch in enumerate(chains)}
                    cur = set()
                    for c in order:
                        if id(c) in first:
                            cur.add(first[id(c)])
                        if id(c) in last:
                            cur.discard(last[id(c)])
                        slices.append(c)
                        opens.append(len(cur))
                    assert opens[-1] == 0
                return slices, opens

            def emit_proj_all(act):
                for sl in proj_slices(act)[0]:
                    sl()

            # ---- body ------------------------------------------------------
            def emit_body(act, nxt):
                accs = act["accs"]
                last_ex = {}

                def emit_unit_front(kb, h):
                    sc_t = ps.tile(
                        [128, 1024], F32, tag="sc", bufs=SC_BUFS, name=f"sc{h}_{kb}"
                    )
                    for c in range(2):
                        nc.tensor.matmul(
                            sc_t[:, c * 512 : (c + 1) * 512],
                            act["kT"][:, kb * 128 : (kb + 1) * 128],
                            act["qT"][
                                :, h * 1024 + c * 512 : h * 1024 + (c + 1) * 512
                            ],
                            start=True,
                            stop=True,
                        )
                    ex = expp.tile([128, 1024], BF16, tag="ex", name=f"ex{h}_{kb}")
                    nc.scalar.activation(ex, sc_t, mybir.ActivationFunctionType.Exp)
                    return ex

                def emit_unit_back(kb, h, ex):
                    for c in range(2):
                        nc.tensor.matmul(
                            oT_ps[:, c * 512 : (c + 1) * 512],
                            act["v"][:, kb],
                            ex[:, c * 512 : (c + 1) * 512],
                            start=kb == 0,
                            stop=kb == KB - 1,
                        )
                    if kb == 0:
                        nc.vector.tensor_copy(out=accs[h], in_=ex)
                    elif kb < KB - 1:
                        nc.vector.tensor_add(accs[h], accs[h], ex)
                    else:
                        last_ex[h] = ex

                def emit_finale(h):
                    # den[q] = ones.T @ (acc + ex_last), replicated across
                    # partitions by the all-ones stationary; den chunks come
                    # from the pp pool so the sc pool stays fronts-only
                    recip = work.tile([128, 1024], F32, tag="recip", name=f"recip{h}")
                    onrm = work.tile([128, 1024], F32, tag="onrm", name=f"onrm{h}")
                    for c in range(2):
                        cc = slice(c * 512, (c + 1) * 512)
                        den_c = ps.tile(
                            [128, 512], F32, tag="pp", bufs=2, name=f"den{h}_{c}"
                        )
                        nc.tensor.matmul(
                            den_c, ones_sb, accs[h][:, cc], start=True, stop=False
                        )
                        nc.tensor.matmul(
                            den_c, ones_sb, last_ex[h][:, cc], start=False, stop=True
                        )
                        nc.vector.reciprocal_approx_fast(out=recip[:, cc], in_=den_c)
                        nc.vector.tensor_mul(onrm[:, cc], oT_ps[:, cc], recip[:, cc])
                        nc.sync.dma_start(
                            out=outT[:, h * 1024 + c * 512 : h * 1024 + (c + 1) * 512],
                            in_=onrm[:, cc],
                        )

                units = [(kb, 0) for kb in range(KB)] + [(kb, 1) for kb in range(KB)]
                if nxt is not None:
                    slices, opens = proj_slices(nxt)
                else:
                    slices, opens = [], []

                pend = []
                fin_pending = []
                st = {"si": 0, "open": False}

                def flush_finales():
                    # only at pp-chain boundaries: emitting den chunks while a
                    # proj chain holds a pp slot deadlocks the in-order PE
                    if not st["open"]:
                        while fin_pending:
                            emit_finale(fin_pending.pop(0))

                def emit_slice():
                    i = st["si"]
                    slices[i]()
                    st["si"] = i + 1
                    st["open"] = opens[i] > 0
                    flush_finales()

                def drain(minpend):
                    while len(pend) > minpend:
                        pkb, ph, pex = pend.pop(0)
                        emit_unit_back(pkb, ph, pex)
                        if pkb == KB - 1:
                            fin_pending.append(ph)
                            flush_finales()

                for ui, (kb, h) in enumerate(units):
                    pend.append((kb, h, emit_unit_front(kb, h)))
                    drain(4)
                    target = ((ui + 1) * len(slices)) // len(units)
                    while st["si"] < target:
                        emit_slice()
                drain(0)
                while st["si"] < len(slices):
                    emit_slice()
                flush_finales()
                assert not fin_pending

            if loop_n is not None:
                assert loop_n % 2 == 0, "pipelined timing loop needs even loop_n"
                actA = alloc_act("A")
                actB = alloc_act("B")
                emit_proj_all(actA)
                with tc.For_i(0, loop_n // 2):
                    emit_body(actA, actB)
                    emit_body(actB, actA)
            elif unroll is not None:
                acts = [alloc_act("A"), alloc_act("B")]
                emit_proj_all(acts[0])
                for i in range(unroll):
                    nxt = acts[(i + 1) % 2] if i + 1 < unroll else None
                    emit_body(acts[i % 2], nxt)
            else:
                actA = alloc_act("A")
                emit_proj_all(actA)
                emit_body(actA, None)

    nc.compile()
    return nc


_NC_CACHE = None


def _get_nc():
    global _NC_CACHE
    if _NC_CACHE is None:
        _NC_CACHE = build_attention_v5()
    return _NC_CACHE


def kernel(x, Wq, Wk, Wv):
    x = np.asarray(x, dtype=np.float32)
    Wq = np.asarray(Wq, dtype=np.float32)
    Wk = np.asarray(Wk, dtype=np.float32)
    Wv = np.asarray(Wv, dtype=np.float32)
    assert x.shape == (B, S, D), x.shape

    bf = ml_dtypes.bfloat16

    def warr(w):
        # [D, H] -> [128p, DC*H] with row p = [chunk0 h..., chunk1 h...]
        return np.ascontiguousarray(
            w.reshape(DC, 128, H).transpose(1, 0, 2).reshape(128, DC * H).astype(bf)
        )

    wq = warr(Wq / np.sqrt(np.float32(H)))
    wk = warr(Wk)
    wv = warr(Wv)
    in_maps = []
    for c in range(8):
        bi, hq = divmod(c, 2)
        xt = x[bi].T  # [d, s]
        if hq == 1:
            xt = np.concatenate([xt[:, SQ:], xt[:, :SQ]], axis=1)
        in_maps.append(
            {
                "xT": np.ascontiguousarray(xt.astype(bf)),
                "wq": wq,
                "wk": wk,
                "wv": wv,
            }
        )

    nc = _get_nc()
    res = run_bass_kernel_spmd(nc, in_maps, core_ids=list(range(8)))

    out = np.empty((B, S, H), dtype=np.float32)
    for c in range(8):
        bi, hq = divmod(c, 2)
        out[bi, hq * SQ : (hq + 1) * SQ] = res.results[c]["outT"].T
    return out


if __name__ == "__main__":
    rng = np.random.default_rng(0)
    x = rng.standard_normal((B, S, D), dtype=np.float32)
    s = 1.0 / np.sqrt(D)
    Wq = rng.standard_normal((D, H), dtype=np.float32) * s
    Wk = rng.standard_normal((D, H), dtype=np.float32) * s
    Wv = rng.standard_normal((D, H), dtype=np.float32) * s
    out = kernel(x=x, Wq=Wq, Wk=Wk, Wv=Wv)
    print("out", out.shape, out.dtype, float(np.abs(out).max()))
